# revision 19
# baseline (speedup 1.0000x reference)
"""ArcFace loss on 8 TRN2 NeuronCores (class-dim tensor parallel), v4.

Per core (classes sharded 8 x 12500, padded to 12512):
    cos[n, c] = e_norm[n, :] @ w_norm[c, :]^T   (fp8 DoubleRow, f32 PSUM)
    then row-sums of exp(64*cos).

v4 vs v3: the v3 single PSUM pool (2 x [128,2048] = all 8 banks) made
every matmul wait for the reader two units back — reader-to-reader
pacing was reader + sem + matmul + sem (~1.3us/2 units of pure stall).
v4 gives each reader stream its OWN PSUM pool (2 x [128,1024] f32 = 2
banks each, 8 banks total) so each stream pipelines independently and
the matmul is fully hidden:
  AA (ACT): activation(Exp, accum_out) does exp AND row-sum in one
      instruction per [128,1024] PSUM tile.
  ZD (DVE): tensor_scalar computes Schraudolph exp bits (p*A + B ->
      int16 RNE; the bit pattern IS bf16 exp(0.25p)) into a batch slab;
      once a slab holds RB units (same row-tile), ONE tensor_scalar
      (bf16 in/out, x1.0, accum_out) sums it — all-SBUF 2-byte operands
      engage the DVE 4x perf mode (~0.26 ns/col vs 1.07 for
      tensor_reduce).
Streams are scheduled t-outer so ZD reduce batches never straddle a
row-tile. The 6.4MB/rep weight DMA is round-robined over the sync and
gpsimd DGE queues.

Schraudolph bias is mean-zero in log space; its ~1.8% rms element
oscillation averages out in the multi-k-term sums (tolerance 2e-2,
v3 measured end-to-end error ~1e-4).

Host: target-class terms exactly in f64 (1024 rows), swaps the margined
target logit into the device sum, masked mean over kept rows.
"""

import numpy as np

N, E, C = 1024, 512, 100000
M = 8            # cores
CSH = C // M     # 12500 classes per core
P = 128
NT = N // P      # 8 batch-row tiles
SCALE = 64.0
MARGIN = 0.5
EPS_NORM = 1e-12
CLAMP = 1e-7

FP8_S = 16.0
ACT_SCALE = SCALE / (FP8_S * FP8_S)          # 0.25
CSHP = 12512                                  # padded classes per core
KP = 2                                        # DoubleRow: 2 contraction chunks
UW = 1024                                     # big-unit width (2 PSUM banks)
TAILW = CSHP - 12 * UW                        # 224

# Schraudolph constants for bf16-bits-as-int16: bits = p*A + B (RNE)
A_SCH = 128.0 * ACT_SCALE * 1.4426950408889634
B_SCH = 128.0 * (127.0 - 0.057304959)         # mean-zero in log space

# column-range widths per stream (sum must be CSHP = 12512); the ~5.6:7.4
# ratio balances ACT (exp+accum ~1.16 ns/col) vs DVE (ts + 4x reduce
# ~1.44 ns/col)
ZD_WIDTHS = [1024] * 5 + [512]
AA_WIDTHS = [1024] * 6 + [512, 224]
RB = 2                                        # ZD reduce batch (slab entries)

_compiled = None
LAST_RESULT = None


def _ranges(zd_w=None, aa_w=None):
    """Column ranges: ZD (DVE Schraudolph) ranges first, then AA (ACT)."""
    zd_w = ZD_WIDTHS if zd_w is None else zd_w
    aa_w = AA_WIDTHS if aa_w is None else aa_w
    assert sum(zd_w) + sum(aa_w) == CSHP
    rs = []
    col = 0
    for w in zd_w:
        rs.append({"g": len(rs), "s": col, "w": w, "job": "ZD"})
        col += w
    for w in aa_w:
        rs.append({"g": len(rs), "s": col, "w": w, "job": "AA"})
        col += w
    return rs


def _zd_batches(zd_r, rb):
    """Group ZD ranges into reduce batches: same width, at most rb per
    batch. Returns per-range (batch, slot, last, bw, bwidth)."""
    out = []
    b = 0
    cur_w, cur_n = None, 0
    for i, r in enumerate(zd_r):
        if cur_w != r["w"] or cur_n == rb:
            if cur_n:
                b += 1
            cur_w, cur_n = r["w"], 0
        out.append({"batch": b, "slot": cur_n})
        cur_n += 1
        nxt = zd_r[i + 1]["w"] if i + 1 < len(zd_r) else None
        if cur_n == rb or nxt != r["w"]:
            for j in range(cur_n):
                out[i - j]["bw"] = cur_n
                out[i - j]["bwidth"] = r["w"]
            out[i]["last"] = True
    for o in out:
        o.setdefault("last", False)
    return out, b + 1


def _schedule(zd_w=None, aa_w=None, rb=RB):
    """Units (range-outer, t-inner per stream) merged by virtual engine
    time so AA (ACT) and ZD (DVE) stay busy throughout; each range's 8
    units finish early so its weight reload for the next rep overlaps."""
    rs = _ranges(zd_w, aa_w)
    aa_r = [r for r in rs if r["job"] == "AA"]
    zd_r = [r for r in rs if r["job"] == "ZD"]
    binfo, nb = _zd_batches(zd_r, rb)
    qa = []
    va = 0.0
    for r in aa_r:
        for t in range(NT):
            va += 0.833 * r["w"] + 330
            qa.append(({"t": t, "job": "AA", **r}, va))
    qz = []
    vz = 0.0
    for zi, r in enumerate(zd_r):
        for t in range(NT):
            vz += 1.302 * r["w"] + 215
            u = {"t": t, "job": "ZD", "zi": zi, **binfo[zi], **r}
            qz.append((u, vz))
    units = []
    ca = cz = 0
    red_t = []
    ia = iz = 0
    while ia < len(qa) or iz < len(qz):
        if iz >= len(qz) or (ia < len(qa) and qa[ia][1] <= qz[iz][1]):
            u = qa[ia][0]
            ia += 1
            u["col"] = ca
            ca += 1
        else:
            u = qz[iz][0]
            iz += 1
            if u["last"]:
                u["rcol"] = cz
                red_t.append(u["t"])
                cz += 1
        units.append(u)
    return rs, units, ca, cz, nb, red_t


RANGES, UNITS, NAA, NRED, NB, RED_T = _schedule()


def _np_in_dtype():
    import concourse.mybir as mybir
    return mybir.dt.np(mybir.dt.float8e4)


def _build(reps=None, zd_w=None, aa_w=None, rb=RB, red_lag=2,
           dma_mode="split2h"):
    import contextlib

    import concourse.mybir as mybir
    import concourse.tile as tile
    from concourse import bacc

    ranges, units, naa, nred, nb, _red_t = _schedule(zd_w, aa_w, rb)

    f32 = mybir.dt.float32
    bf16 = mybir.dt.bfloat16
    i16 = mybir.dt.int16
    fin = mybir.dt.float8e4
    EXP = mybir.ActivationFunctionType.Exp
    MULT, ADD = mybir.AluOpType.mult, mybir.AluOpType.add
    DR = mybir.MatmulPerfMode.DoubleRow

    nc = bacc.Bacc("TRN2", target_bir_lowering=False, debug=False, num_devices=M)
    et_d = nc.dram_tensor("et", [KP, P, 2, N], fin, kind="ExternalInput").ap()
    wt_d = nc.dram_tensor("wt", [KP, P, 2, CSHP], fin, kind="ExternalInput").ap()
    outa_d = nc.dram_tensor("outa", [P, max(naa, 1)], f32, kind="ExternalOutput").ap()
    outz_d = nc.dram_tensor("outz", [P, max(nred, 1)], f32, kind="ExternalOutput").ap()

    with tile.TileContext(nc) as tc:
        with tc.tile_pool(name="wp", bufs=1) as wp, \
             tc.tile_pool(name="ep", bufs=1) as ep, \
             tc.tile_pool(name="etp", bufs=2) as etp, \
             tc.tile_pool(name="slp", bufs=1) as slp, \
             tc.tile_pool(name="dmp", bufs=2) as dmp, \
             tc.tile_pool(name="dup", bufs=2) as dup, \
             tc.tile_pool(name="stp", bufs=2) as stp, \
             tc.tile_pool(name="psa", bufs=2, space="PSUM") as ppa, \
             tc.tile_pool(name="psz", bufs=2, space="PSUM") as ppz, \
             (tc.For_i(0, reps, 1,
                       hint_engines=(mybir.EngineType.PE,
                                     mybir.EngineType.Activation))
              if reps else contextlib.nullcontext()):
            # warm the Exp table off the critical path
            warm = ep.tile([P, 1], f32, tag="warm", name="warm")
            nc.vector.memset(warm[:], 0.0)
            warm2 = ep.tile([P, 1], f32, tag="warm2", name="warm2")
            nc.scalar.activation(warm2[:], warm[:], EXP, scale=1.0)

            # weight DMA through multiple DGE queues
            halved = dma_mode == "split2h"
            dma_engs = {"single": [nc.sync],
                        "split2": [nc.sync, nc.gpsimd],
                        "split2h": [nc.sync, nc.gpsimd],
                        "none": None}[dma_mode]

            def load(t, src, di=[0]):
                if dma_engs is None:
                    nc.gpsimd.memset(t[:], 0.0)
                    return
                if halved:
                    h = t.shape[-1] // 2
                    for dst_h, src_h in ((t[:, :, :h], src[:, :, :h]),
                                         (t[:, :, h:], src[:, :, h:])):
                        e = dma_engs[di[0] % len(dma_engs)]
                        di[0] += 1
                        e.dma_start(dst_h, src_h)
                else:
                    e = dma_engs[di[0] % len(dma_engs)]
                    di[0] += 1
                    e.dma_start(t[:], src)

            et = []
            for k in range(KP):
                t = etp.tile([P, 2, N], fin, tag=f"et{k}", name=f"et{k}")
                load(t, et_d[k])
                et.append(t)
            # issue weight DMAs in first-use order so the head of the rep
            # isn't blocked behind ranges used later
            first_use = []
            seen = set()
            for u in units:
                if u["g"] not in seen:
                    seen.add(u["g"])
                    first_use.append(u["g"])
            wt = {}
            for g in first_use:
                r = ranges[g]
                tk = []
                for k in range(KP):
                    t = wp.tile([P, 2, r["w"]], fin, tag=f"w{g}_{k}",
                                name=f"w{g}_{k}")
                    load(t, wt_d[k, :, :, r["s"]:r["s"] + r["w"]])
                    tk.append(t)
                wt[g] = tk

            stats_a = stp.tile([P, max(naa, 1)], f32, tag="sta", name="sta")
            stats_z = stp.tile([P, max(nred, 1)], f32, tag="stz", name="stz")
            if nred == 0:
                nc.vector.memset(stats_z[:], 0.0)
            if naa == 0:
                nc.vector.memset(stats_a[:], 0.0)

            # per-(row-tile, batch) Schraudolph slabs; batches alternate
            # between two persistent buffers per tile (batch b+1's first
            # write only needs batch b-1's reduce done)
            slabs = {}
            for t in range(NT):
                for b in range(nb):
                    key = (t, b % 2)
                    if key not in slabs:
                        slabs[key] = slp.tile([P, rb, UW], i16,
                                              tag=f"slab{t}_{b % 2}",
                                              name=f"slab{t}_{b % 2}")
                    slabs[(t, b)] = slabs[key]

            pending = []  # (due_unit_idx, emit_fn) delayed DVE reduces
            for ui, u in enumerate(units):
                while pending and pending[0][0] <= ui:
                    pending.pop(0)[1]()
                t, w, job = u["t"], u["w"], u["job"]
                pp = ppa if job == "AA" else ppz
                ps = pp.tile([P, UW], f32, tag="ps", name=f"ps{job}{ui}")
                nsub = (w + 511) // 512
                for j in range(nsub):
                    jw = min(512, w - j * 512)
                    for k in range(KP):
                        nc.tensor.matmul(
                            ps[:, j * 512:j * 512 + jw],
                            et[k][:, :, t * P:(t + 1) * P],
                            wt[u["g"]][k][:, :, j * 512:j * 512 + jw],
                            start=(k == 0), stop=(k == KP - 1),
                            perf_mode=DR,
                        )
                if job == "AA":
                    dump = dmp.tile([P, UW], bf16, tag="dump", name=f"dump{ui}")
                    nc.scalar.activation(
                        dump[:, :w], ps[:, :w], EXP, scale=ACT_SCALE,
                        accum_out=stats_a[:, u["col"]:u["col"] + 1])
                else:  # ZD
                    slab = slabs[(t, u["batch"])]
                    sl = u["slot"]
                    nc.vector.tensor_scalar(
                        slab[:, sl:sl + 1, :w], ps[:, :w], A_SCH, B_SCH,
                        MULT, ADD)
                    if u["last"]:
                        def emit(slab=slab, k=u["bw"], bw=u["bwidth"],
                                 rcol=u["rcol"]):
                            dummy = dup.tile([P, rb, UW], bf16, tag="dummy",
                                             name=f"dm{rcol}")
                            nc.vector.tensor_scalar(
                                dummy[:, :k, :bw],
                                slab[:, :k, :bw].bitcast(bf16),
                                1.0, 0.0, MULT, ADD,
                                accum_out=stats_z[:, rcol:rcol + 1])
                        if red_lag == 0:
                            emit()
                        else:
                            pending.append((ui + red_lag, emit))
            for _, fn in pending:
                fn()

            nc.sync.dma_start(outa_d[:, :], stats_a[:])
            nc.sync.dma_start(outz_d[:, :], stats_z[:])

    nc.compile()
    return nc


def _prep_operands(e, w):
    """Normalize rows, pre-scale, quantize, lay out [KP, P, 2, cols]."""
    dt = _np_in_dtype()
    s = FP8_S
    wn = (w * (s / np.maximum(np.sqrt(np.einsum('ij,ij->i', w, w)), EPS_NORM))[:, None]).astype(dt)
    en = (e * (s / np.maximum(np.sqrt(np.einsum('ij,ij->i', e, e)), EPS_NORM))[:, None]).astype(dt)

    def lay(xT, cols):  # xT: [E, cols] -> [KP, P, 2, cols]
        return np.ascontiguousarray(
            xT.reshape(KP, 2, P, cols).transpose(0, 2, 1, 3))

    et_arr = lay(np.ascontiguousarray(en.T), N)
    shards = []
    for i in range(M):
        blk = wn[i * CSH:(i + 1) * CSH]
        bT = np.zeros((E, CSHP), dt)
        bT[:, :CSH] = blk.T
        shards.append(lay(bT, CSHP))
    return et_arr, shards


def kernel(embedding, ground_truth, weight):
    global _compiled, LAST_RESULT
    import os
    os.environ["BASS_NEVER_TRACE"] = "1"
    from concourse.bass_utils import run_bass_kernel_spmd

    e = np.ascontiguousarray(np.asarray(embedding, dtype=np.float32))
    w = np.ascontiguousarray(np.asarray(weight, dtype=np.float32))
    gt = np.asarray(ground_truth).astype(np.int64)

    et_arr, shards = _prep_operands(e, w)
    in_maps = [{"et": et_arr, "wt": shards[i]} for i in range(M)]

    if _compiled is None:
        _compiled = _build()
    LAST_RESULT = run_bass_kernel_spmd(_compiled, in_maps, core_ids=list(range(M)))

    # ---- host combine (f64) ----
    # stats_a col -> row-tile t of the col-th AA unit; stats_z col -> the
    # slab's row-tile (slabs are per-(t, batch), never straddle t).
    aa_t = np.asarray([u["t"] for u in UNITS if u["job"] == "AA"])
    red_t = np.asarray(RED_T)
    S = np.zeros(N, np.float64)
    for r in LAST_RESULT.results:
        oa = r["outa"].astype(np.float64)
        oz = r["outz"].astype(np.float64)
        for t in range(NT):
            sl = slice(t * P, (t + 1) * P)
            S[sl] += oa[:, aa_t == t].sum(axis=1)
            S[sl] += oz[:, red_t == t].sum(axis=1)
    S -= float(M * (CSHP - CSH))   # zero-pad cols: exp(0)=1 each, in AA units

    # exact target-class terms
    e64 = e.astype(np.float64)
    en64 = e64 / np.maximum(np.sqrt((e64 * e64).sum(1, keepdims=True)), EPS_NORM)
    wg = w[gt].astype(np.float64)
    wg /= np.maximum(np.sqrt((wg * wg).sum(1, keepdims=True)), EPS_NORM)
    cos_gt = np.clip((en64 * wg).sum(1), -1.0 + CLAMP, 1.0 - CLAMP)
    keep = (np.arccos(cos_gt) + MARGIN) <= np.pi
    tgt = SCALE * (cos_gt * np.cos(MARGIN) - np.sqrt(1.0 - cos_gt * cos_gt) * np.sin(MARGIN))

    S_corr = S - np.exp(SCALE * cos_gt) + np.exp(tgt)
    nll = np.log(S_corr) - tgt
    loss = (nll * keep).sum() / max(keep.sum(), 1.0)
    return np.float32(loss)


# revision 45
# speedup vs baseline: 1.0072x; 1.0072x over previous
"""ArcFace loss on 8 TRN2 NeuronCores (class-dim tensor parallel), v4.

Per core (classes sharded 8 x 12500, padded to 12512):
    cos[n, c] = e_norm[n, :] @ w_norm[c, :]^T   (fp8 DoubleRow, f32 PSUM)
    then row-sums of exp(64*cos).

v4 vs v3: the v3 single PSUM pool (2 x [128,2048] = all 8 banks) made
every matmul wait for the reader two units back — reader-to-reader
pacing was reader + sem + matmul + sem (~1.3us/2 units of pure stall).
v4 gives each reader stream its OWN PSUM pool (2 x [128,1024] f32 = 2
banks each, 8 banks total) so each stream pipelines independently and
the matmul is fully hidden:
  AA (ACT): activation(Exp, accum_out) does exp AND row-sum in one
      instruction per [128,1024] PSUM tile.
  ZD (DVE): tensor_scalar computes Schraudolph exp bits (p*A + B ->
      int16 RNE; the bit pattern IS bf16 exp(0.25p)) into a batch slab;
      once a slab holds RB units (same row-tile), ONE tensor_scalar
      (bf16 in/out, x1.0, accum_out) sums it — all-SBUF 2-byte operands
      engage the DVE 4x perf mode (~0.26 ns/col vs 1.07 for
      tensor_reduce).
Streams are scheduled t-outer so ZD reduce batches never straddle a
row-tile. The 6.4MB/rep weight DMA is round-robined over the sync and
gpsimd DGE queues.

Schraudolph bias is mean-zero in log space; its ~1.8% rms element
oscillation averages out in the multi-k-term sums (tolerance 2e-2,
v3 measured end-to-end error ~1e-4).

Host: target-class terms exactly in f64 (1024 rows), swaps the margined
target logit into the device sum, masked mean over kept rows.
"""

import numpy as np

N, E, C = 1024, 512, 100000
M = 8            # cores
CSH = C // M     # 12500 classes per core
P = 128
NT = N // P      # 8 batch-row tiles
SCALE = 64.0
MARGIN = 0.5
EPS_NORM = 1e-12
CLAMP = 1e-7

FP8_S = 16.0
ACT_SCALE = SCALE / (FP8_S * FP8_S)          # 0.25
CSHP = 12512                                  # padded classes per core
KP = 2                                        # DoubleRow: 2 contraction chunks
UW = 1024                                     # big-unit width (2 PSUM banks)
TAILW = CSHP - 12 * UW                        # 224

# Schraudolph constants for bf16-bits-as-int16: bits = p*A + B (RNE)
A_SCH = 128.0 * ACT_SCALE * 1.4426950408889634
B_SCH = 128.0 * (127.0 - 0.057304959)         # mean-zero in log space

# column-range widths per stream (sum must be CSHP = 12512); the ~5.6:7.4
# ratio balances ACT (exp+accum ~1.16 ns/col) vs DVE (ts + 4x reduce
# ~1.44 ns/col)
ZD_WIDTHS = [1024] * 5 + [512]
AA_WIDTHS = [1024] * 6 + [512, 224]
FE_WIDTHS = []
RB = 2                                        # ZD reduce batch (slab entries)
SWI = False   # DoubleRowSwInterleave stationary layout (contiguous ldweights)

_compiled = None
LAST_RESULT = None


def _ranges(zd_w=None, aa_w=None, fe_w=None):
    """Column ranges: ZD (DVE Schraudolph), FE (flipped, PE ones-reduce),
    then AA (ACT exp+accum; tail pad cols must stay in AA)."""
    zd_w = ZD_WIDTHS if zd_w is None else zd_w
    aa_w = AA_WIDTHS if aa_w is None else aa_w
    fe_w = FE_WIDTHS if fe_w is None else fe_w
    assert sum(zd_w) + sum(aa_w) + sum(fe_w) == CSHP
    assert all(w % 128 == 0 for w in fe_w)
    rs = []
    col = 0
    for w in zd_w:
        rs.append({"g": len(rs), "s": col, "w": w, "job": "ZD"})
        col += w
    for w in fe_w:
        rs.append({"g": len(rs), "s": col, "w": w, "job": "FE"})
        col += w
    for w in aa_w:
        rs.append({"g": len(rs), "s": col, "w": w, "job": "AA"})
        col += w
    return rs


def _zd_batches(zd_r, rb):
    """Group ZD ranges into reduce batches: same width, at most rb per
    batch. Returns per-range (batch, slot, last, bw, bwidth)."""
    out = []
    b = 0
    cur_w, cur_n = None, 0
    for i, r in enumerate(zd_r):
        if cur_w != r["w"] or cur_n == rb:
            if cur_n:
                b += 1
            cur_w, cur_n = r["w"], 0
        out.append({"batch": b, "slot": cur_n})
        cur_n += 1
        nxt = zd_r[i + 1]["w"] if i + 1 < len(zd_r) else None
        if cur_n == rb or nxt != r["w"]:
            for j in range(cur_n):
                out[i - j]["bw"] = cur_n
                out[i - j]["bwidth"] = r["w"]
            out[i]["last"] = True
    for o in out:
        o.setdefault("last", False)
    return out, b + 1


def _schedule(zd_w=None, aa_w=None, rb=RB, fe_w=None):
    """Units (range-outer, t-inner per stream) merged by virtual engine
    time so AA+FE (ACT) and ZD (DVE) stay busy throughout; each range's
    units finish early so its weight reload for the next rep overlaps.
    FE units are flipped 128-class chunks covering all N at once; they
    share the ACT stream (exp) and are reduced by PE ones-matmuls."""
    rs = _ranges(zd_w, aa_w, fe_w)
    aa_r = [r for r in rs if r["job"] == "AA"]
    zd_r = [r for r in rs if r["job"] == "ZD"]
    fe_r = [r for r in rs if r["job"] == "FE"]
    binfo, nb = _zd_batches(zd_r, rb)
    qa = []
    va = 0.0
    fe_units = []
    for r in fe_r:
        for c in range(r["w"] // 128):
            fe_units.append({"job": "FE", "c": c, **r})
    # spread FE chunks among AA units on the shared ACT virtual clock
    aa_units = []
    for r in aa_r:
        for t in range(NT):
            aa_units.append({"t": t, "job": "AA", **r})
    na, nf = len(aa_units), len(fe_units)
    ifa = iff = 0
    mixed = []
    while ifa < na or iff < nf:
        take_aa = iff >= nf or (ifa < na and ifa * nf <= iff * na)
        if take_aa:
            mixed.append(aa_units[ifa]); ifa += 1
        else:
            mixed.append(fe_units[iff]); iff += 1
    for u in mixed:
        # HW-calibrated per-unit engine times (ACT exp+accum from PSUM
        # ~1.06 ns/col + 390; FE exp ~1.06*1024 + 200)
        va += (1.06 * u["w"] + 390) if u["job"] == "AA" else (1.06 * 1024 + 200)
        qa.append((u, va))
    for i, u in enumerate(fe_units):
        u["fe_first"] = (i == 0)
        u["fe_last"] = (i == nf - 1)
    qz = []
    vz = 0.0
    for zi, r in enumerate(zd_r):
        for t in range(NT):
            # HW: DVE ts from PSUM ~2.2 ns/col + 190, + ~0.3/col reduce share
            vz += 2.5 * r["w"] + 190
            u = {"t": t, "job": "ZD", "zi": zi, **binfo[zi], **r}
            qz.append((u, vz))
    units = []
    ca = cz = 0
    red_t = []
    ia = iz = 0
    while ia < len(qa) or iz < len(qz):
        if iz >= len(qz) or (ia < len(qa) and qa[ia][1] <= qz[iz][1]):
            u = qa[ia][0]
            ia += 1
            if u["job"] == "AA":
                u["col"] = ca
                ca += 1
        else:
            u = qz[iz][0]
            iz += 1
            if u["last"]:
                u["rcol"] = cz
                red_t.append(u["t"])
                cz += 1
        units.append(u)
    return rs, units, ca, cz, nb, red_t


RANGES, UNITS, NAA, NRED, NB, RED_T = _schedule()


def _np_in_dtype():
    import concourse.mybir as mybir
    return mybir.dt.np(mybir.dt.float8e4)


def _build(reps=None, zd_w=None, aa_w=None, rb=RB, red_lag=2,
           dma_mode="split2h", red_mode="ts4x", mm_order="jk", mm_w=512,
           probe_narrow=0, psa_bufs=2, psz_bufs=2, psw=UW, swi=None):
    import contextlib

    import concourse.mybir as mybir
    import concourse.tile as tile
    from concourse import bacc

    swi = SWI if swi is None else swi
    ranges, units, naa, nred, nb, _red_t = _schedule(zd_w, aa_w, rb)

    f32 = mybir.dt.float32
    bf16 = mybir.dt.bfloat16
    i16 = mybir.dt.int16
    fin = mybir.dt.float8e4
    EXP = mybir.ActivationFunctionType.Exp
    MULT, ADD = mybir.AluOpType.mult, mybir.AluOpType.add
    DR = mybir.MatmulPerfMode.DoubleRow
    DRS = mybir.MatmulPerfMode.DoubleRowSwInterleave

    nc = bacc.Bacc("TRN2", target_bir_lowering=False, debug=False, num_devices=M)
    et_shape = [KP, P, NT * 2, 128] if swi else [KP, P, 2, N]
    et_d = nc.dram_tensor("et", et_shape, fin, kind="ExternalInput").ap()
    wt_d = nc.dram_tensor("wt", [KP, P, 2, CSHP], fin, kind="ExternalInput").ap()
    outa_d = nc.dram_tensor("outa", [P, max(naa, 1)], f32, kind="ExternalOutput").ap()
    outz_d = nc.dram_tensor("outz", [P, max(nred, 1)], f32, kind="ExternalOutput").ap()

    with tile.TileContext(nc) as tc:
        with tc.tile_pool(name="wp", bufs=1) as wp, \
             tc.tile_pool(name="ep", bufs=1) as ep, \
             tc.tile_pool(name="etp", bufs=2) as etp, \
             tc.tile_pool(name="slp", bufs=1) as slp, \
             tc.tile_pool(name="dmp", bufs=2) as dmp, \
             tc.tile_pool(name="dup", bufs=2) as dup, \
             tc.tile_pool(name="stp", bufs=2) as stp, \
             tc.tile_pool(name="psa", bufs=psa_bufs, space="PSUM") as ppa, \
             tc.tile_pool(name="psz", bufs=psz_bufs, space="PSUM") as ppz, \
             (tc.For_i(0, reps, 1,
                       hint_engines=(mybir.EngineType.PE,
                                     mybir.EngineType.Activation))
              if reps else contextlib.nullcontext()):
            # warm the Exp table off the critical path
            warm = ep.tile([P, 1], f32, tag="warm", name="warm")
            nc.vector.memset(warm[:], 0.0)
            warm2 = ep.tile([P, 1], f32, tag="warm2", name="warm2")
            nc.scalar.activation(warm2[:], warm[:], EXP, scale=1.0)

            # weight DMA through multiple DGE queues
            halved = dma_mode == "split2h"
            dma_engs = {"single": [nc.sync],
                        "split2": [nc.sync, nc.gpsimd],
                        "split2h": [nc.sync, nc.gpsimd],
                        "none": None,
                        "skip": "skip"}[dma_mode]

            def load(t, src, di=[0]):
                if dma_engs == "skip":
                    return
                if dma_engs is None:
                    nc.gpsimd.memset(t[:], 0.0)
                    return
                if halved:
                    h = t.shape[-1] // 2
                    for dst_h, src_h in ((t[:, :, :h], src[:, :, :h]),
                                         (t[:, :, h:], src[:, :, h:])):
                        e = dma_engs[di[0] % len(dma_engs)]
                        di[0] += 1
                        e.dma_start(dst_h, src_h)
                else:
                    e = dma_engs[di[0] % len(dma_engs)]
                    di[0] += 1
                    e.dma_start(t[:], src)

            et = []
            for k in range(KP):
                t = etp.tile(et_shape[1:], fin, tag=f"et{k}", name=f"et{k}")
                load(t, et_d[k])
                et.append(t)
            # issue weight DMAs in first-use order so the head of the rep
            # isn't blocked behind ranges used later
            first_use = []
            seen = set()
            for u in units:
                if u["g"] not in seen:
                    seen.add(u["g"])
                    first_use.append(u["g"])
            wt = {}
            for g in first_use:
                r = ranges[g]
                tk = []
                for k in range(KP):
                    t = wp.tile([P, 2, r["w"]], fin, tag=f"w{g}_{k}",
                                name=f"w{g}_{k}")
                    load(t, wt_d[k, :, :, r["s"]:r["s"] + r["w"]])
                    tk.append(t)
                wt[g] = tk

            stats_a = stp.tile([P, max(naa, 1)], f32, tag="sta", name="sta")
            stats_z = stp.tile([P, max(nred, 1)], f32, tag="stz", name="stz")
            if nred == 0:
                nc.vector.memset(stats_z[:], 0.0)
            if naa == 0:
                nc.vector.memset(stats_a[:], 0.0)

            # per-(row-tile, batch) Schraudolph slabs; batches alternate
            # between two persistent buffers per tile (batch b+1's first
            # write only needs batch b-1's reduce done)
            slabs = {}
            for t in range(NT):
                for b in range(nb):
                    key = (t, b % 2)
                    if key not in slabs:
                        slabs[key] = slp.tile([P, rb, UW], i16,
                                              tag=f"slab{t}_{b % 2}",
                                              name=f"slab{t}_{b % 2}")
                    slabs[(t, b)] = slabs[key]

            pending = []  # (due_unit_idx, emit_fn) delayed DVE reduces
            for ui, u in enumerate(units):
                while pending and pending[0][0] <= ui:
                    pending.pop(0)[1]()
                t, w, job = u["t"], u["w"], u["job"]
                pp = ppa if job == "AA" else ppz
                ps = pp.tile([P, psw], f32, tag="ps", name=f"ps{job}{ui}")
                nsub = (w + mm_w - 1) // mm_w
                if mm_order == "jk":
                    mm_iter = [(j, k) for j in range(nsub) for k in range(KP)]
                else:  # "kj": stationary et[k] loaded once per k
                    mm_iter = [(j, k) for k in range(KP) for j in range(nsub)]
                for j, k in mm_iter:
                    jw = min(mm_w, w - j * mm_w)
                    lhs = (et[k][:, 2 * t:2 * t + 2, :] if swi
                           else et[k][:, :, t * P:(t + 1) * P])
                    nc.tensor.matmul(
                        ps[:, j * mm_w:j * mm_w + jw],
                        lhs,
                        wt[u["g"]][k][:, :, j * mm_w:j * mm_w + jw],
                        start=(k == 0), stop=(k == KP - 1),
                        perf_mode=(DRS if swi else DR),
                    )
                if job == "AA":
                    rw = min(probe_narrow, w) if probe_narrow else w
                    dump = dmp.tile([P, UW], bf16, tag="dump", name=f"dump{ui}")
                    nc.scalar.activation(
                        dump[:, :rw], ps[:, :rw], EXP, scale=ACT_SCALE,
                        accum_out=stats_a[:, u["col"]:u["col"] + 1])
                else:  # ZD
                    slab = slabs[(t, u["batch"])]
                    sl = u["slot"]
                    nc.vector.tensor_scalar(
                        slab[:, sl:sl + 1, :w], ps[:, :w], A_SCH, B_SCH,
                        MULT, ADD)
                    if u["last"]:
                        def emit(slab=slab, k=u["bw"], bw=u["bwidth"],
                                 rcol=u["rcol"]):
                            if red_mode == "none":
                                return
                            if red_mode == "reduce":
                                nc.vector.reduce_sum(
                                    stats_z[:, rcol:rcol + 1],
                                    slab[:, :k, :bw].bitcast(bf16),
                                    axis=mybir.AxisListType.XY)
                                return
                            dummy = dup.tile([P, rb, UW], bf16, tag="dummy",
                                             name=f"dm{rcol}")
                            nc.vector.tensor_scalar(
                                dummy[:, :k, :bw],
                                slab[:, :k, :bw].bitcast(bf16),
                                1.0, 0.0, MULT, ADD,
                                accum_out=stats_z[:, rcol:rcol + 1])
                        if red_lag == 0:
                            emit()
                        else:
                            pending.append((ui + red_lag, emit))
            for _, fn in pending:
                fn()

            nc.sync.dma_start(outa_d[:, :], stats_a[:])
            nc.sync.dma_start(outz_d[:, :], stats_z[:])

    nc.compile()
    return nc


def _prep_operands(e, w, swi=None):
    """Normalize rows, pre-scale, quantize, lay out [KP, P, 2, cols].
    swi: stationary embeddings in DoubleRowSwInterleave layout — per
    partition and row-tile, [A127,B127,...,A0,B0] (interleaved k-pair,
    reversed stationary column), as [KP, P, NT*2, 128]."""
    swi = SWI if swi is None else swi
    dt = _np_in_dtype()
    s = FP8_S
    wn = (w * (s / np.maximum(np.sqrt(np.einsum('ij,ij->i', w, w)), EPS_NORM))[:, None]).astype(dt)
    en = (e * (s / np.maximum(np.sqrt(np.einsum('ij,ij->i', e, e)), EPS_NORM))[:, None]).astype(dt)

    def lay(xT, cols):  # xT: [E, cols] -> [KP, P, 2, cols]
        return np.ascontiguousarray(
            xT.reshape(KP, 2, P, cols).transpose(0, 2, 1, 3))

    et_arr = lay(np.ascontiguousarray(en.T), N)
    if swi:
        et_arr = np.ascontiguousarray(
            et_arr.reshape(KP, P, 2, NT, 128)[..., ::-1]
            .transpose(0, 1, 3, 4, 2).reshape(KP, P, NT * 2, 128))
    shards = []
    for i in range(M):
        blk = wn[i * CSH:(i + 1) * CSH]
        bT = np.zeros((E, CSHP), dt)
        bT[:, :CSH] = blk.T
        shards.append(lay(bT, CSHP))
    return et_arr, shards


def kernel(embedding, ground_truth, weight):
    global _compiled, LAST_RESULT
    import os
    os.environ["BASS_NEVER_TRACE"] = "1"
    from concourse.bass_utils import run_bass_kernel_spmd

    e = np.ascontiguousarray(np.asarray(embedding, dtype=np.float32))
    w = np.ascontiguousarray(np.asarray(weight, dtype=np.float32))
    gt = np.asarray(ground_truth).astype(np.int64)

    et_arr, shards = _prep_operands(e, w)
    in_maps = [{"et": et_arr, "wt": shards[i]} for i in range(M)]

    if _compiled is None:
        _compiled = _build()
    LAST_RESULT = run_bass_kernel_spmd(_compiled, in_maps, core_ids=list(range(M)))

    # ---- host combine (f64) ----
    # stats_a col -> row-tile t of the col-th AA unit; stats_z col -> the
    # slab's row-tile (slabs are per-(t, batch), never straddle t).
    aa_t = np.asarray([u["t"] for u in UNITS if u["job"] == "AA"])
    red_t = np.asarray(RED_T)
    S = np.zeros(N, np.float64)
    for r in LAST_RESULT.results:
        oa = r["outa"].astype(np.float64)
        oz = r["outz"].astype(np.float64)
        for t in range(NT):
            sl = slice(t * P, (t + 1) * P)
            S[sl] += oa[:, aa_t == t].sum(axis=1)
            S[sl] += oz[:, red_t == t].sum(axis=1)
    S -= float(M * (CSHP - CSH))   # zero-pad cols: exp(0)=1 each, in AA units

    # exact target-class terms
    e64 = e.astype(np.float64)
    en64 = e64 / np.maximum(np.sqrt((e64 * e64).sum(1, keepdims=True)), EPS_NORM)
    wg = w[gt].astype(np.float64)
    wg /= np.maximum(np.sqrt((wg * wg).sum(1, keepdims=True)), EPS_NORM)
    cos_gt = np.clip((en64 * wg).sum(1), -1.0 + CLAMP, 1.0 - CLAMP)
    keep = (np.arccos(cos_gt) + MARGIN) <= np.pi
    tgt = SCALE * (cos_gt * np.cos(MARGIN) - np.sqrt(1.0 - cos_gt * cos_gt) * np.sin(MARGIN))

    S_corr = S - np.exp(SCALE * cos_gt) + np.exp(tgt)
    nll = np.log(S_corr) - tgt
    loss = (nll * keep).sum() / max(keep.sum(), 1.0)
    return np.float32(loss)
